# revision 1
# baseline (speedup 1.0000x reference)
"""Bass/Tile kernel for nn_BasicGRUClassifier on 8 Trainium2 NeuronCores.

Strategy (data-parallel over batch, 32 samples/core):
  All on-chip tensors use [H=128 partitions, B=32 free] layout -> zero transposes.
  Phase 1: L0 input projections Wx0^T x_t for all t via big weight-stationary
           matmuls (chunked over time, pipelined into phase 2).
  Phase 2: L0 recurrence, 281 steps: PSUM prefilled with the precomputed x-part
           via an identity matmul (makes the add free), 3 gate matmuls
           accumulate on top, ACT does sigmoid/tanh straight out of PSUM,
           DVE/GPSIMD do the elementwise lerp.
  Phase 3: L1 input projections from the stored h0 sequence (chunked, batched).
  Phase 4: L1 recurrence (identical cell structure to phase 2).
  Phase 5: FC classifier: h1_final stationary, Wfc streamed; bias via a K=1
           ones-row matmul into the same PSUM accumulation group.
"""

import numpy as np

HID = 128
IN_CH = 271
SEQ = 281
NCLS = 1854
BATCH = 256
NCORES = 8
BL = BATCH // NCORES  # 32 per-core batch
LCH = 16              # timesteps per chunk
G3 = 3 * HID          # 384

_CACHE = {}


def _build(seq_t):
    import concourse.bass as bass
    import concourse.bacc as bacc
    import concourse.tile as tile
    import concourse.mybir as mybir
    from concourse.masks import make_identity

    fp32 = mybir.dt.float32
    AF = mybir.ActivationFunctionType

    nch = (seq_t + LCH - 1) // LCH

    nc = bacc.Bacc()
    X = nc.dram_tensor("X", [BL, IN_CH, seq_t], fp32, kind="ExternalInput")
    Wx0 = nc.dram_tensor("Wx0", [IN_CH, G3], fp32, kind="ExternalInput")
    Uh0 = nc.dram_tensor("Uh0", [HID, G3], fp32, kind="ExternalInput")
    Wx1 = nc.dram_tensor("Wx1", [HID, G3], fp32, kind="ExternalInput")
    Uh1 = nc.dram_tensor("Uh1", [HID, G3], fp32, kind="ExternalInput")
    B0 = nc.dram_tensor("B0", [HID, 3], fp32, kind="ExternalInput")
    B1 = nc.dram_tensor("B1", [HID, 3], fp32, kind="ExternalInput")
    WFC = nc.dram_tensor("WFC", [HID, NCLS], fp32, kind="ExternalInput")
    BFC = nc.dram_tensor("BFC", [1, NCLS], fp32, kind="ExternalInput")
    OUT = nc.dram_tensor("OUT", [BL, NCLS], fp32, kind="ExternalOutput")

    with tile.TileContext(nc) as tc:
        from contextlib import ExitStack
        with ExitStack() as ctx:
            const = ctx.enter_context(tc.tile_pool(name="const", bufs=1))
            x0p = ctx.enter_context(tc.tile_pool(name="x0p", bufs=3))
            h0p = ctx.enter_context(tc.tile_pool(name="h0p", bufs=3))
            x1p = ctx.enter_context(tc.tile_pool(name="x1p", bufs=3))
            recs = ctx.enter_context(tc.tile_pool(name="recs", bufs=4))
            hs = ctx.enter_context(tc.tile_pool(name="hs", bufs=3))
            outp = ctx.enter_context(tc.tile_pool(name="outp", bufs=1))
            ps1 = ctx.enter_context(tc.tile_pool(name="ps1", bufs=3, space="PSUM"))
            psr = ctx.enter_context(tc.tile_pool(name="psr", bufs=4, space="PSUM"))

            # ---- constants into SBUF ----
            ident = const.tile([HID, HID], fp32)
            make_identity(nc, ident)

            ksz = [128, 128, IN_CH - 256]  # K-tiles of the 271-dim input
            wx0_sb = []
            for k in range(3):
                t_ = const.tile([ksz[k], G3], fp32, tag=f"wx0_{k}")
                nc.sync.dma_start(out=t_, in_=Wx0[sum(ksz[:k]):sum(ksz[:k]) + ksz[k], :])
                wx0_sb.append(t_)
            uh0_sb = const.tile([HID, G3], fp32, tag="uh0")
            nc.sync.dma_start(out=uh0_sb, in_=Uh0[:, :])
            wx1_sb = const.tile([HID, G3], fp32, tag="wx1")
            nc.sync.dma_start(out=wx1_sb, in_=Wx1[:, :])
            uh1_sb = const.tile([HID, G3], fp32, tag="uh1")
            nc.sync.dma_start(out=uh1_sb, in_=Uh1[:, :])
            b0_sb = const.tile([HID, 3], fp32, tag="b0")
            nc.sync.dma_start(out=b0_sb, in_=B0[:, :])
            b1_sb = const.tile([HID, 3], fp32, tag="b1")
            nc.sync.dma_start(out=b1_sb, in_=B1[:, :])
            wfc_sb = const.tile([HID, NCLS], fp32, tag="wfc")
            nc.sync.dma_start(out=wfc_sb, in_=WFC[:, :])
            bfc_sb = const.tile([1, NCLS], fp32, tag="bfc")
            nc.sync.dma_start(out=bfc_sb, in_=BFC[:, :])
            ones_sb = const.tile([1, BL], fp32, tag="ones")
            nc.vector.memset(ones_sb, 1.0)

            # X^T resident tiles: [c_tile, BL*seq_t], col = b*seq_t + t.
            # DMA rows are contiguous seq_t*4B runs.
            xt_sb = []
            for k in range(3):
                t_ = const.tile([ksz[k], BL * seq_t], fp32, tag=f"xt_{k}")
                c0 = sum(ksz[:k])
                src = X[:, c0:c0 + ksz[k], :].rearrange("b c t -> c b t")
                dst = t_.rearrange("c (b t) -> c b t", b=BL)
                nc.sync.dma_start(out=dst, in_=src)
                xt_sb.append(t_)

            # initial states (zero)
            h0_init = const.tile([HID, BL], fp32, tag="h0i")
            nc.vector.memset(h0_init, 0.0)
            h1_init = const.tile([HID, BL], fp32, tag="h1i")
            nc.vector.memset(h1_init, 0.0)

            chlen = [min(LCH, seq_t - c * LCH) for c in range(nch)]

            # ---------- emit helpers ----------
            def phase1(ch):
                """L0 x-projection for chunk ch -> x0 tiles [128, 32*Lc],
                col = b*Lc + tl."""
                Lc = chlen[ch]
                t0 = ch * LCH
                tiles = []
                for g in range(3):
                    ps = ps1.tile([HID, BL * LCH], fp32, tag="ps1")
                    for k in range(3):
                        rhs = xt_sb[k].rearrange("c (b t) -> c b t", b=BL)[:, :, t0:t0 + Lc]
                        nc.tensor.matmul(
                            ps[:, : BL * Lc],
                            wx0_sb[k][:, g * HID:(g + 1) * HID],
                            rhs,
                            start=(k == 0),
                            stop=(k == 2),
                        )
                    xt_ = x0p.tile([HID, BL * LCH], fp32, tag=f"x0_{g}")
                    nc.scalar.activation(
                        xt_[:, : BL * Lc], ps[:, : BL * Lc], AF.Identity,
                        bias=b0_sb[:, g:g + 1],
                    )
                    tiles.append(xt_)
                return tiles

            def phase3(ch, h0seq):
                """L1 x-projection for chunk ch from h0seq [128, 32*Lc]
                (col = tl*32 + b) -> x1 tiles, same layout."""
                Lc = chlen[ch]
                tiles = []
                for g in range(3):
                    ps = ps1.tile([HID, BL * LCH], fp32, tag="ps1")
                    nc.tensor.matmul(
                        ps[:, : BL * Lc],
                        wx1_sb[:, g * HID:(g + 1) * HID],
                        h0seq[:, : BL * Lc],
                        start=True, stop=True,
                    )
                    xt_ = x1p.tile([HID, BL * LCH], fp32, tag=f"x1_{g}")
                    nc.scalar.activation(
                        xt_[:, : BL * Lc], ps[:, : BL * Lc], AF.Identity,
                        bias=b1_sb[:, g:g + 1],
                    )
                    tiles.append(xt_)
                return tiles

            def cell(h_prev, xr, xu, xo, uh, h_out):
                """One GRU cell in [H, B] layout. xr/xu/xo are [128, BL] APs
                (may be strided). h_out: [128, BL] AP to write new state."""
                ps = psr.tile([HID, 96], fp32, tag="psr")
                # start=True clears has_written for the WHOLE bank -> only the
                # first prefill may use it; later writes to untouched regions
                # still overwrite (bit clear) and set bits for accumulation.
                nc.tensor.matmul(ps[:, 0:32], ident, xr, start=True, stop=False)
                nc.tensor.matmul(ps[:, 32:64], ident, xu, start=False, stop=False)
                nc.tensor.matmul(ps[:, 64:96], ident, xo, start=False, stop=False)
                nc.tensor.matmul(ps[:, 0:32], uh[:, 0:HID], h_prev, start=False, stop=True)
                nc.tensor.matmul(ps[:, 32:64], uh[:, HID:2 * HID], h_prev, start=False, stop=True)
                ru = recs.tile([HID, 64], fp32, tag="ru")
                nc.scalar.activation(ru, ps[:, 0:64], AF.Sigmoid)
                rh = recs.tile([HID, BL], fp32, tag="rh")
                nc.vector.tensor_mul(rh, ru[:, 0:32], h_prev)
                nc.tensor.matmul(ps[:, 64:96], uh[:, 2 * HID:3 * HID], rh, start=False, stop=True)
                o = recs.tile([HID, BL], fp32, tag="o")
                nc.scalar.activation(o, ps[:, 64:96], AF.Tanh)
                d = recs.tile([HID, BL], fp32, tag="d")
                nc.vector.tensor_sub(d, o, h_prev)
                e = recs.tile([HID, BL], fp32, tag="e")
                nc.gpsimd.tensor_mul(e, ru[:, 32:64], d)
                nc.vector.tensor_add(h_out, h_prev, e)

            # ---------- main pipeline ----------
            x0_tiles = {}
            x1_tiles = {}
            h0seq_tiles = {}
            x0_tiles[0] = phase1(0)
            if nch > 1:
                x0_tiles[1] = phase1(1)

            h0_cur = h0_init[:, :]
            h1_state = [h1_init[:, :]]

            def l1_step(tg):
                pch, tl = tg // LCH, tg % LCH
                xr1, xu1, xo1 = x1_tiles[pch]
                h1_new = hs.tile([HID, BL], fp32, tag="h1")
                cell(h1_state[0],
                     xr1[:, tl * BL:(tl + 1) * BL],
                     xu1[:, tl * BL:(tl + 1) * BL],
                     xo1[:, tl * BL:(tl + 1) * BL],
                     uh1_sb, h1_new)
                h1_state[0] = h1_new[:, :]

            for ch in range(nch):
                Lc = chlen[ch]
                if ch + 2 < nch:
                    x0_tiles[ch + 2] = phase1(ch + 2)
                h0seq = h0p.tile([HID, BL * LCH], fp32, tag="h0seq")
                h0seq_tiles[ch] = h0seq
                xr0, xu0, xo0 = x0_tiles[ch]
                for tl in range(Lc):
                    tg = ch * LCH + tl
                    xav = [x[:, :BL * Lc].rearrange("p (b t) -> p b t", b=BL)[:, :, tl]
                           for x in (xr0, xu0, xo0)]
                    h_out = h0seq[:, tl * BL:(tl + 1) * BL]
                    cell(h0_cur, xav[0], xav[1], xav[2], uh0_sb, h_out)
                    h0_cur = h_out
                    if tg - LCH >= 0:
                        l1_step(tg - LCH)
                x0_tiles.pop(ch)
                x1_tiles[ch] = phase3(ch, h0seq)
                h0seq_tiles.pop(ch)

            # tail: remaining L1 steps
            for tg in range(max(0, seq_t - LCH), seq_t):
                l1_step(tg)
            h1_cur = h1_state[0]

            # ---------- FC ----------
            out_sb = outp.tile([BL, NCLS], fp32, tag="osb")
            nsl = [512, 512, 512, NCLS - 3 * 512]
            for i in range(4):
                n0 = i * 512
                ps = ps1.tile([HID, BL * LCH], fp32, tag="ps1")
                pf = ps[:BL, : nsl[i]]
                nc.tensor.matmul(pf, ones_sb, bfc_sb[:, n0:n0 + nsl[i]], start=True, stop=False)
                nc.tensor.matmul(pf, h1_cur, wfc_sb[:, n0:n0 + nsl[i]], start=False, stop=True)
                nc.scalar.activation(out_sb[:, n0:n0 + nsl[i]], pf, AF.Identity, bias=0.0)
            nc.sync.dma_start(out=OUT[:, :], in_=out_sb)

    nc.finalize()
    return nc


def _prep_consts(inputs):
    f32 = np.float32
    Wx0 = np.ascontiguousarray(
        np.concatenate([inputs["Wr0"][:IN_CH], inputs["Wu0"][:IN_CH], inputs["Wo0"][:IN_CH]], axis=1), f32)
    Uh0 = np.ascontiguousarray(
        np.concatenate([inputs["Wr0"][IN_CH:], inputs["Wu0"][IN_CH:], inputs["Wo0"][IN_CH:]], axis=1), f32)
    Wx1 = np.ascontiguousarray(
        np.concatenate([inputs["Wr1"][:HID], inputs["Wu1"][:HID], inputs["Wo1"][:HID]], axis=1), f32)
    Uh1 = np.ascontiguousarray(
        np.concatenate([inputs["Wr1"][HID:], inputs["Wu1"][HID:], inputs["Wo1"][HID:]], axis=1), f32)
    B0 = np.ascontiguousarray(np.stack([inputs["br0"], inputs["bu0"], inputs["bo0"]], axis=1), f32)
    B1 = np.ascontiguousarray(np.stack([inputs["br1"], inputs["bu1"], inputs["bo1"]], axis=1), f32)
    WFC = np.ascontiguousarray(inputs["Wfc"], f32)
    BFC = np.ascontiguousarray(inputs["bfc"][None, :], f32)
    return dict(Wx0=Wx0, Uh0=Uh0, Wx1=Wx1, Uh1=Uh1, B0=B0, B1=B1, WFC=WFC, BFC=BFC)


def kernel(_trace=False, **inputs):
    from concourse.bass_utils import run_bass_kernel_spmd

    seq_t = inputs["X"].shape[2]
    if "nc" not in _CACHE or _CACHE.get("seq_t") != seq_t:
        _CACHE["nc"] = _build(seq_t)
        _CACHE["seq_t"] = seq_t
    nc = _CACHE["nc"]

    consts = _prep_consts(inputs)
    X = np.ascontiguousarray(inputs["X"], np.float32)
    in_maps = []
    for c in range(NCORES):
        m = dict(consts)
        m["X"] = np.ascontiguousarray(X[c * BL:(c + 1) * BL])
        in_maps.append(m)

    res = run_bass_kernel_spmd(nc, in_maps, core_ids=list(range(NCORES)), trace=_trace)
    out = np.concatenate([r["OUT"] for r in res.results], axis=0)
    if _trace:
        _CACHE["last_exec_time_ns"] = res.exec_time_ns
        _CACHE["last_profile"] = res.profile_json
    return out



# revision 3
# speedup vs baseline: 2.6842x; 2.6842x over previous
"""Bass/Tile kernel for nn_BasicGRUClassifier on 8 Trainium2 NeuronCores.

Strategy (data-parallel over batch, 32 samples/core, bf16 matmul datapath):
  All on-chip tensors use [H=128 partitions, B=32 free] layout, t-major
  PSUM gate banks (col = tl*32 + b) so every critical-path access is
  contiguous.

  Per chunk of LCH=8 timesteps and per layer, three PSUM banks hold the
  pre-activation gate values:
    RU bank [128, 512]: r at cols 0:256, u at cols 256:512
    O  bank [128, 512]: layer0 o at 0:256, layer1 o at 256:512 (shared)
  The banks are seeded by bias matmuls (K=1 against a ones row) plus the
  batched x-projection matmuls; the recurrent U@h matmuls then accumulate
  into per-step 32-col slices, so no identity-prefill matmuls and no
  PSUM->SBUF gate copies are needed.

  Cell update is restructured as
    m = (u-1)*h          (gpsimd, off critical path)
    e = u*o              (vector)
    h' = e - m           (vector; = (1-u)h + u*o)
  so a single vector op separates tanh from the next step's matmuls.

  Everything the PE touches is bf16 (fp32 matmuls double-pump the PE:
  2x LDWEIGHTS + 2x MATMUL per instruction). PSUM accumulation stays
  fp32; activations read fp32 PSUM and emit bf16. Verified numerically:
  bf16 end-to-end rel err vs fp32 reference = 3.9e-3 (tolerance 2e-2).
"""

import numpy as np
import ml_dtypes

HID = 128
IN_CH = 271
SEQ = 281
NCLS = 1854
BATCH = 256
NCORES = 8
BL = BATCH // NCORES  # 32 per-core batch
LCH = 8               # timesteps per chunk (PSUM bank = 512 fp32 = 2*LCH*BL)
G3 = 3 * HID

_CACHE = {}


def _build(seq_t):
    import concourse.bacc as bacc
    import concourse.tile as tile
    import concourse.mybir as mybir
    from contextlib import ExitStack

    fp32 = mybir.dt.float32
    bf16 = mybir.dt.bfloat16
    AF = mybir.ActivationFunctionType
    ALU = mybir.AluOpType

    nch = (seq_t + LCH - 1) // LCH
    chlen = [min(LCH, seq_t - c * LCH) for c in range(nch)]

    nc = bacc.Bacc()
    XT = nc.dram_tensor("XT", [IN_CH, seq_t * BL], bf16, kind="ExternalInput")
    WX0 = nc.dram_tensor("WX0", [IN_CH, G3], bf16, kind="ExternalInput")
    UH0 = nc.dram_tensor("UH0", [HID, G3], bf16, kind="ExternalInput")
    WX1 = nc.dram_tensor("WX1", [HID, G3], bf16, kind="ExternalInput")
    UH1 = nc.dram_tensor("UH1", [HID, G3], bf16, kind="ExternalInput")
    B0R = nc.dram_tensor("B0R", [1, G3], bf16, kind="ExternalInput")
    B1R = nc.dram_tensor("B1R", [1, G3], bf16, kind="ExternalInput")
    WFC = nc.dram_tensor("WFC", [HID, NCLS], bf16, kind="ExternalInput")
    BFC = nc.dram_tensor("BFC", [1, NCLS], bf16, kind="ExternalInput")
    OUT = nc.dram_tensor("OUT", [BL, NCLS], fp32, kind="ExternalOutput")

    ksz = [128, 128, IN_CH - 256]

    with tile.TileContext(nc) as tc:
        with ExitStack() as ctx:
            const = ctx.enter_context(tc.tile_pool(name="const", bufs=1))
            seqp = ctx.enter_context(tc.tile_pool(name="seqp", bufs=2))
            cellp = ctx.enter_context(tc.tile_pool(name="cellp", bufs=4))
            outp = ctx.enter_context(tc.tile_pool(name="outp", bufs=1))
            ru0ps = ctx.enter_context(tc.tile_pool(name="ru0ps", bufs=2, space="PSUM"))
            ru1ps = ctx.enter_context(tc.tile_pool(name="ru1ps", bufs=2, space="PSUM"))
            obps = ctx.enter_context(tc.tile_pool(name="obps", bufs=3, space="PSUM"))
            fcps = ctx.enter_context(tc.tile_pool(name="fcps", bufs=1, space="PSUM"))

            # ---- constants into SBUF ----
            xt_sb = []
            for k in range(3):
                t_ = const.tile([ksz[k], seq_t * BL], bf16, tag=f"xt{k}")
                c0 = sum(ksz[:k])
                nc.sync.dma_start(out=t_, in_=XT[c0:c0 + ksz[k], :])
                xt_sb.append(t_)
            wx0_sb = []
            for k in range(3):
                t_ = const.tile([ksz[k], G3], bf16, tag=f"wx0{k}")
                c0 = sum(ksz[:k])
                nc.sync.dma_start(out=t_, in_=WX0[c0:c0 + ksz[k], :])
                wx0_sb.append(t_)
            uh0_sb = const.tile([HID, G3], bf16, tag="uh0")
            nc.sync.dma_start(out=uh0_sb, in_=UH0[:, :])
            wx1_sb = const.tile([HID, G3], bf16, tag="wx1")
            nc.sync.dma_start(out=wx1_sb, in_=WX1[:, :])
            uh1_sb = const.tile([HID, G3], bf16, tag="uh1")
            nc.sync.dma_start(out=uh1_sb, in_=UH1[:, :])
            b0_sb = const.tile([1, G3], bf16, tag="b0")
            nc.sync.dma_start(out=b0_sb, in_=B0R[:, :])
            b1_sb = const.tile([1, G3], bf16, tag="b1")
            nc.sync.dma_start(out=b1_sb, in_=B1R[:, :])
            wfc_sb = const.tile([HID, NCLS], bf16, tag="wfc")
            nc.sync.dma_start(out=wfc_sb, in_=WFC[:, :])
            bfc_sb = const.tile([1, NCLS], bf16, tag="bfc")
            nc.sync.dma_start(out=bfc_sb, in_=BFC[:, :])
            ones_sb = const.tile([1, LCH * BL], bf16, tag="ones")
            nc.vector.memset(ones_sb, 1.0)
            h0i = const.tile([HID, BL], bf16, tag="h0i")
            nc.vector.memset(h0i, 0.0)
            h1i = const.tile([HID, BL], bf16, tag="h1i")
            nc.vector.memset(h1i, 0.0)

            ru0_bank = {}
            ru1_bank = {}
            o_bank = {}
            h0seq = {}

            def phase_l0(c):
                """Seed chunk c's RU0/O banks: bias + batched x-projection."""
                Lc = chlen[c]
                n = Lc * BL
                t0 = c * LCH * BL
                ru = ru0ps.tile([HID, 2 * LCH * BL], fp32, tag="ru0")
                ob = obps.tile([HID, 2 * LCH * BL], fp32, tag="ob")
                ru0_bank[c] = ru
                o_bank[c] = ob
                nc.tensor.matmul(ru[:, 0:n], b0_sb[:, 0:HID], ones_sb[:, 0:n],
                                 start=True, stop=False)
                nc.tensor.matmul(ru[:, 256:256 + n], b0_sb[:, HID:2 * HID],
                                 ones_sb[:, 0:n], start=False, stop=False)
                nc.tensor.matmul(ob[:, 0:n], b0_sb[:, 2 * HID:G3],
                                 ones_sb[:, 0:n], start=True, stop=False)
                for g, dst in ((0, ru[:, 0:n]), (1, ru[:, 256:256 + n]),
                               (2, ob[:, 0:n])):
                    for k in range(3):
                        nc.tensor.matmul(
                            dst, wx0_sb[k][:, g * HID:(g + 1) * HID],
                            xt_sb[k][:, t0:t0 + n], start=False, stop=False)

            def phase_l1_bias(c):
                """L1 bias seed for chunk c (no h0seq dependency)."""
                Lc = chlen[c]
                n = Lc * BL
                ru = ru1ps.tile([HID, 2 * LCH * BL], fp32, tag="ru1")
                ru1_bank[c] = ru
                nc.tensor.matmul(ru[:, 0:n], b1_sb[:, 0:HID], ones_sb[:, 0:n],
                                 start=True, stop=False)
                nc.tensor.matmul(ru[:, 256:256 + n], b1_sb[:, HID:2 * HID],
                                 ones_sb[:, 0:n], start=False, stop=False)
                nc.tensor.matmul(o_bank[c][:, 256:256 + n],
                                 b1_sb[:, 2 * HID:G3], ones_sb[:, 0:n],
                                 start=False, stop=False)

            def phase_l1_proj(c):
                """L1 x-projection for chunk c from the completed h0seq."""
                Lc = chlen[c]
                n = Lc * BL
                ru = ru1_bank[c]
                hs = h0seq[c]
                nc.tensor.matmul(ru[:, 0:n], wx1_sb[:, 0:HID], hs[:, 0:n],
                                 start=False, stop=False)
                nc.tensor.matmul(ru[:, 256:256 + n], wx1_sb[:, HID:2 * HID],
                                 hs[:, 0:n], start=False, stop=False)
                nc.tensor.matmul(o_bank[c][:, 256:256 + n],
                                 wx1_sb[:, 2 * HID:G3], hs[:, 0:n],
                                 start=False, stop=False)

            def cell(layer, c, tl, h_prev, h_out):
                """One GRU cell; returns AP of the new state (== h_out)."""
                if layer == 0:
                    ru_bank, uh, ooff = ru0_bank[c], uh0_sb, 0
                else:
                    ru_bank, uh, ooff = ru1_bank[c], uh1_sb, 256
                ob = o_bank[c]
                s = tl * BL
                nc.tensor.matmul(ru_bank[:, s:s + BL], uh[:, 0:HID], h_prev,
                                 start=False, stop=True)
                nc.tensor.matmul(ru_bank[:, 256 + s:256 + s + BL],
                                 uh[:, HID:2 * HID], h_prev,
                                 start=False, stop=True)
                # layer1's plain elementwise ops ride on GpSimd to keep the
                # DVE free for layer0's critical chain
                ew = nc.vector if layer == 0 else nc.gpsimd
                ru_t = cellp.tile([HID, 2 * BL], bf16, tag=f"ru{layer}t")
                nc.scalar.activation(
                    ru_t.rearrange("p (g x) -> p g x", g=2),
                    ru_bank.rearrange("p (g x) -> p g x", g=2)[:, :, s:s + BL],
                    AF.Sigmoid)
                rh = cellp.tile([HID, BL], bf16, tag=f"rh{layer}")
                ew.tensor_mul(rh, ru_t[:, 0:BL], h_prev)
                m = cellp.tile([HID, BL], bf16, tag=f"m{layer}")
                nc.vector.scalar_tensor_tensor(
                    m, ru_t[:, BL:2 * BL], 1.0, h_prev,
                    op0=ALU.subtract, op1=ALU.mult)
                nc.tensor.matmul(ob[:, ooff + s:ooff + s + BL],
                                 uh[:, 2 * HID:G3], rh, start=False, stop=True)
                o_t = cellp.tile([HID, BL], bf16, tag=f"o{layer}")
                nc.scalar.activation(o_t, ob[:, ooff + s:ooff + s + BL], AF.Tanh)
                e = cellp.tile([HID, BL], bf16, tag=f"e{layer}")
                ew.tensor_mul(e, ru_t[:, BL:2 * BL], o_t)
                ew.tensor_sub(h_out, e, m)
                return h_out

            # ---------- main pipeline ----------
            phase_l0(0)
            h0_cur = h0i[:, :]
            h1_cur = h1i[:, :]
            next_l1 = 0

            def l1_step(tg):
                nonlocal h1_cur, next_l1
                c1, tl1 = divmod(tg, LCH)
                h1_new = cellp.tile([HID, BL], bf16, tag="h1s")
                h1_cur = cell(1, c1, tl1, h1_cur, h1_new[:, :])
                next_l1 = tg + 1

            for c in range(nch):
                phase_l1_bias(c)
                if c + 1 < nch:
                    phase_l0(c + 1)
                hs = seqp.tile([HID, LCH * BL], bf16, tag="h0seq")
                h0seq[c] = hs
                for tl in range(chlen[c]):
                    h0_cur = cell(0, c, tl, h0_cur, hs[:, tl * BL:(tl + 1) * BL])
                    tg1 = c * LCH + tl - LCH
                    if tg1 >= 0:
                        l1_step(tg1)
                phase_l1_proj(c)

            for tg1 in range(next_l1, seq_t):
                l1_step(tg1)

            # ---------- FC ----------
            out_sb = outp.tile([BL, NCLS], fp32, tag="osb")
            nsl = [512, 512, 512, NCLS - 3 * 512]
            for i in range(4):
                n0 = i * 512
                fc = fcps.tile([BL, 512], fp32, tag="fc")
                pf = fc[:, 0:nsl[i]]
                nc.tensor.matmul(pf, ones_sb[:, 0:BL], bfc_sb[:, n0:n0 + nsl[i]],
                                 start=True, stop=False)
                nc.tensor.matmul(pf, h1_cur, wfc_sb[:, n0:n0 + nsl[i]],
                                 start=False, stop=True)
                nc.scalar.activation(out_sb[:, n0:n0 + nsl[i]], pf, AF.Identity)
            nc.sync.dma_start(out=OUT[:, :], in_=out_sb)

    nc.finalize()
    return nc


def _prep_consts(inputs):
    bf = ml_dtypes.bfloat16
    Wx0 = np.ascontiguousarray(np.concatenate(
        [inputs["Wr0"][:IN_CH], inputs["Wu0"][:IN_CH], inputs["Wo0"][:IN_CH]],
        axis=1).astype(bf))
    Uh0 = np.ascontiguousarray(np.concatenate(
        [inputs["Wr0"][IN_CH:], inputs["Wu0"][IN_CH:], inputs["Wo0"][IN_CH:]],
        axis=1).astype(bf))
    Wx1 = np.ascontiguousarray(np.concatenate(
        [inputs["Wr1"][:HID], inputs["Wu1"][:HID], inputs["Wo1"][:HID]],
        axis=1).astype(bf))
    Uh1 = np.ascontiguousarray(np.concatenate(
        [inputs["Wr1"][HID:], inputs["Wu1"][HID:], inputs["Wo1"][HID:]],
        axis=1).astype(bf))
    B0R = np.ascontiguousarray(np.concatenate(
        [inputs["br0"], inputs["bu0"], inputs["bo0"]])[None, :].astype(bf))
    B1R = np.ascontiguousarray(np.concatenate(
        [inputs["br1"], inputs["bu1"], inputs["bo1"]])[None, :].astype(bf))
    WFC = np.ascontiguousarray(inputs["Wfc"].astype(bf))
    BFC = np.ascontiguousarray(inputs["bfc"][None, :].astype(bf))
    return dict(WX0=Wx0, UH0=Uh0, WX1=Wx1, UH1=Uh1, B0R=B0R, B1R=B1R,
                WFC=WFC, BFC=BFC)


def kernel(_trace=False, **inputs):
    from concourse.bass_utils import run_bass_kernel_spmd

    seq_t = inputs["X"].shape[2]
    if "nc" not in _CACHE or _CACHE.get("seq_t") != seq_t:
        _CACHE["nc"] = _build(seq_t)
        _CACHE["seq_t"] = seq_t
    nc = _CACHE["nc"]

    consts = _prep_consts(inputs)
    bf = ml_dtypes.bfloat16
    # [B, C, T] -> per-core [C, T, BL] (t-major columns: col = t*BL + b)
    X = inputs["X"].astype(bf)
    in_maps = []
    for c in range(NCORES):
        m = dict(consts)
        xc = X[c * BL:(c + 1) * BL].transpose(1, 2, 0)  # [C, T, BL]
        m["XT"] = np.ascontiguousarray(xc).reshape(IN_CH, seq_t * BL)
        in_maps.append(m)

    res = run_bass_kernel_spmd(nc, in_maps, core_ids=list(range(NCORES)),
                               trace=_trace)
    out = np.concatenate([r["OUT"] for r in res.results], axis=0)
    if _trace:
        _CACHE["last_exec_time_ns"] = res.exec_time_ns
        _CACHE["last_profile"] = res.profile_json
    return out


# revision 9
# speedup vs baseline: 2.7846x; 1.0374x over previous
"""Bass/Tile kernel for nn_BasicGRUClassifier on 8 Trainium2 NeuronCores.

Strategy (data-parallel over batch, 32 samples/core, bf16 matmul datapath):
  All on-chip tensors use [H=128 partitions, B=32 free] layout, t-major
  PSUM gate banks (col = tl*32 + b) so every critical-path access is
  contiguous.

  Per chunk of LCH=16 timesteps, PSUM holds the pre-activation gates:
    RU0 [128,1024] (2 banks, double-buffered): L0 r at 0:512, u at 512:1024
    RU1 [128,1024] (2 banks, single buffer):   L1 r / u
    O0  [128,512]  (1 bank):  L0 o            O1 [128,512] (1 bank): L1 o
  Banks are seeded by the batched x-projection matmuls (L0 biases ride a
  ones-channel appended to X's last K-tile; L1 r/u biases are K=1 matmuls
  against a ones row; L1 o bias comes in through the tanh bias operand).
  The recurrent U@h matmuls accumulate into per-step 32-col slices, so no
  identity-prefill matmuls and no PSUM->SBUF gate copies are needed.

  Cell update is restructured as
    m = (u-1)*h          (off critical path)
    e = u*o
    h' = e - m           (= (1-u)h + u*o)
  so little work separates tanh from the next step's matmuls.

  Everything the PE touches is bf16 (fp32 matmuls double-pump the PE:
  2x LDWEIGHTS + 2x MATMUL per instruction). PSUM accumulation stays
  fp32; activations read fp32 PSUM and emit bf16. Verified numerically:
  bf16 end-to-end rel err vs fp32 reference = 3.9e-3 (tolerance 2e-2).
"""

import numpy as np
import ml_dtypes

HID = 128
IN_CH = 271
SEQ = 281
NCLS = 1854
BATCH = 256
NCORES = 8
BL = BATCH // NCORES  # 32 per-core batch
LCH = 16              # timesteps per chunk
G3 = 3 * HID
RUW = 2 * LCH * BL    # 1024: r/u region stride inside a RU tile

_CACHE = {}


def _build(seq_t):
    import concourse.bacc as bacc
    import concourse.tile as tile
    import concourse.mybir as mybir
    from contextlib import ExitStack

    fp32 = mybir.dt.float32
    bf16 = mybir.dt.bfloat16
    AF = mybir.ActivationFunctionType
    ALU = mybir.AluOpType

    nch = (seq_t + LCH - 1) // LCH
    chlen = [min(LCH, seq_t - c * LCH) for c in range(nch)]
    UOFF = LCH * BL  # 512: u-region offset (bank B of a RU tile)

    nc = bacc.Bacc()
    XT = nc.dram_tensor("XT", [IN_CH, seq_t * BL], bf16, kind="ExternalInput")
    WX0 = nc.dram_tensor("WX0", [IN_CH + 1, G3], bf16, kind="ExternalInput")
    UH0 = nc.dram_tensor("UH0", [HID, G3], bf16, kind="ExternalInput")
    WX1 = nc.dram_tensor("WX1", [HID, G3], bf16, kind="ExternalInput")
    UH1 = nc.dram_tensor("UH1", [HID, G3], bf16, kind="ExternalInput")
    B1R = nc.dram_tensor("B1R", [1, G3], bf16, kind="ExternalInput")
    B1O = nc.dram_tensor("B1O", [HID, 1], fp32, kind="ExternalInput")
    WFC = nc.dram_tensor("WFC", [HID, NCLS], bf16, kind="ExternalInput")
    BFC = nc.dram_tensor("BFC", [1, NCLS], bf16, kind="ExternalInput")
    OUT = nc.dram_tensor("OUT", [BL, NCLS], fp32, kind="ExternalOutput")

    ksz = [128, 128, IN_CH - 256 + 1]  # third tile: 15 channels + ones row

    with tile.TileContext(nc) as tc:
        with ExitStack() as ctx:
            const = ctx.enter_context(tc.tile_pool(name="const", bufs=1))
            seqp = ctx.enter_context(tc.tile_pool(name="seqp", bufs=2))
            cellp = ctx.enter_context(tc.tile_pool(name="cellp", bufs=4))
            outp = ctx.enter_context(tc.tile_pool(name="outp", bufs=1))
            ru0ps = ctx.enter_context(tc.tile_pool(name="ru0ps", bufs=2, space="PSUM"))
            ru1ps = ctx.enter_context(tc.tile_pool(name="ru1ps", bufs=1, space="PSUM"))
            o0ps = ctx.enter_context(tc.tile_pool(name="o0ps", bufs=1, space="PSUM"))
            o1ps = ctx.enter_context(tc.tile_pool(name="o1ps", bufs=1, space="PSUM"))

            # ---- constants into SBUF ----
            xt_sb = []
            for k in range(3):
                t_ = const.tile([ksz[k], seq_t * BL], bf16, tag=f"xt{k}")
                c0 = sum(ksz[:k])
                if k < 2:
                    nc.sync.dma_start(out=t_, in_=XT[c0:c0 + ksz[k], :])
                else:
                    # row 15 is the ones-channel that carries the L0 biases;
                    # fill the tile with 1.0, then overlay the 15 real rows
                    nc.vector.memset(t_, 1.0)
                    nc.sync.dma_start(out=t_[0:15, :], in_=XT[256:271, :])
                xt_sb.append(t_)
            wx0_sb = []
            for k in range(3):
                t_ = const.tile([ksz[k], G3], bf16, tag=f"wx0{k}")
                c0 = sum(ksz[:k])
                nc.sync.dma_start(out=t_, in_=WX0[c0:c0 + ksz[k], :])
                wx0_sb.append(t_)
            uh0_sb = const.tile([HID, G3], bf16, tag="uh0")
            nc.sync.dma_start(out=uh0_sb, in_=UH0[:, :])
            wx1_sb = const.tile([HID, G3], bf16, tag="wx1")
            nc.sync.dma_start(out=wx1_sb, in_=WX1[:, :])
            uh1_sb = const.tile([HID, G3], bf16, tag="uh1")
            nc.sync.dma_start(out=uh1_sb, in_=UH1[:, :])
            b1_sb = const.tile([1, G3], bf16, tag="b1")
            nc.sync.dma_start(out=b1_sb, in_=B1R[:, :])
            b1o_sb = const.tile([HID, 1], fp32, tag="b1o")
            nc.sync.dma_start(out=b1o_sb, in_=B1O[:, :])
            wfc_sb = const.tile([HID, NCLS], bf16, tag="wfc")
            nc.sync.dma_start(out=wfc_sb, in_=WFC[:, :])
            bfc_sb = const.tile([1, NCLS], bf16, tag="bfc")
            nc.sync.dma_start(out=bfc_sb, in_=BFC[:, :])
            ones_sb = const.tile([1, LCH * BL], bf16, tag="ones")
            nc.vector.memset(ones_sb, 1.0)
            h0i = const.tile([HID, BL], bf16, tag="h0i")
            nc.vector.memset(h0i, 0.0)
            h1i = const.tile([HID, BL], bf16, tag="h1i")
            nc.vector.memset(h1i, 0.0)

            ru0_bank = {}
            ru1_bank = {}
            o0_bank = {}
            o1_bank = {}
            h0seq = {}

            # Deferrable phase matmuls are queued as closures and drained a
            # couple per cell, so they fill PE idle gaps mid-chunk instead of
            # serializing at chunk boundaries ahead of critical cell matmuls.
            pending = []

            def drain_pending(k=2):
                for _ in range(min(k, len(pending))):
                    pending.pop(0)()

            def phase_l0_ru(c, defer=True):
                """Chunk c's L0 r/u pre-activations: batched x-projection
                (biases ride the ones-channel in xt_sb[2])."""
                n = chlen[c] * BL
                t0 = c * LCH * BL
                ru = ru0ps.tile([HID, RUW], fp32, tag="ru0")
                ru0_bank[c] = ru
                for g, off in ((0, 0), (1, UOFF)):
                    for k in range(3):
                        def mm(g=g, off=off, k=k):
                            nc.tensor.matmul(
                                ru[:, off:off + n],
                                wx0_sb[k][:, g * HID:(g + 1) * HID],
                                xt_sb[k][:, t0:t0 + n],
                                start=(k == 0), stop=False)
                        if defer:
                            pending.append(mm)
                        else:
                            mm()

            def phase_l0_o(c):
                """Chunk c's L0 o pre-activation (single-buffered bank, so
                emitted at the start of chunk c, not prefetched)."""
                n = chlen[c] * BL
                t0 = c * LCH * BL
                ob = o0ps.tile([HID, LCH * BL], fp32, tag="o0")
                o0_bank[c] = ob
                for k in range(3):
                    nc.tensor.matmul(
                        ob[:, 0:n], wx0_sb[k][:, 2 * HID:G3],
                        xt_sb[k][:, t0:t0 + n], start=(k == 0), stop=False)

            def phase_l1(c):
                """L1 bias + x-projection for chunk c from completed h0seq.
                Queued (not emitted inline) so the matmuls spread into the
                next chunk's PE gaps."""
                n = chlen[c] * BL
                ru = ru1ps.tile([HID, RUW], fp32, tag="ru1")
                ob = o1ps.tile([HID, LCH * BL], fp32, tag="o1")
                ru1_bank[c] = ru
                o1_bank[c] = ob
                hs = h0seq[c]
                mms = [
                    lambda: nc.tensor.matmul(
                        ru[:, 0:n], b1_sb[:, 0:HID], ones_sb[:, 0:n],
                        start=True, stop=False),
                    lambda: nc.tensor.matmul(
                        ru[:, UOFF:UOFF + n], b1_sb[:, HID:2 * HID],
                        ones_sb[:, 0:n], start=True, stop=False),
                    lambda: nc.tensor.matmul(
                        ru[:, 0:n], wx1_sb[:, 0:HID], hs[:, 0:n],
                        start=False, stop=False),
                    lambda: nc.tensor.matmul(
                        ru[:, UOFF:UOFF + n], wx1_sb[:, HID:2 * HID],
                        hs[:, 0:n], start=False, stop=False),
                    lambda: nc.tensor.matmul(
                        ob[:, 0:n], wx1_sb[:, 2 * HID:G3], hs[:, 0:n],
                        start=True, stop=False),
                ]
                pending.extend(mms)

            def cell(layer, c, tl, h_prev, h_out):
                """One GRU cell; returns AP of the new state (== h_out)."""
                if layer == 0:
                    ru_bank, ob, uh = ru0_bank[c], o0_bank[c], uh0_sb
                else:
                    ru_bank, ob, uh = ru1_bank[c], o1_bank[c], uh1_sb
                s = tl * BL
                nc.tensor.matmul(ru_bank[:, s:s + BL], uh[:, 0:HID], h_prev,
                                 start=False, stop=True)
                nc.tensor.matmul(ru_bank[:, UOFF + s:UOFF + s + BL],
                                 uh[:, HID:2 * HID], h_prev,
                                 start=False, stop=True)
                # layer1's plain elementwise ops ride on GpSimd to keep the
                # DVE free for layer0's critical chain
                ew = nc.vector if layer == 0 else nc.gpsimd
                ru_t = cellp.tile([HID, 2 * BL], bf16, tag=f"ru{layer}t")
                nc.scalar.activation(
                    ru_t.rearrange("p (g x) -> p g x", g=2),
                    ru_bank.rearrange("p (g x) -> p g x", g=2)[:, :, s:s + BL],
                    AF.Sigmoid)
                rh = cellp.tile([HID, BL], bf16, tag=f"rh{layer}")
                ew.tensor_mul(rh, ru_t[:, 0:BL], h_prev)
                m = cellp.tile([HID, BL], bf16, tag=f"m{layer}")
                nc.vector.scalar_tensor_tensor(
                    m, ru_t[:, BL:2 * BL], 1.0, h_prev,
                    op0=ALU.subtract, op1=ALU.mult)
                nc.tensor.matmul(ob[:, s:s + BL], uh[:, 2 * HID:G3], rh,
                                 start=False, stop=True)
                o_t = cellp.tile([HID, BL], bf16, tag=f"o{layer}")
                if layer == 0:
                    nc.scalar.activation(o_t, ob[:, s:s + BL], AF.Tanh)
                else:
                    nc.scalar.activation(o_t, ob[:, s:s + BL], AF.Tanh,
                                         bias=b1o_sb[:, 0:1])
                e = cellp.tile([HID, BL], bf16, tag=f"e{layer}")
                ew.tensor_mul(e, ru_t[:, BL:2 * BL], o_t)
                ew.tensor_sub(h_out, e, m)
                return h_out

            # ---------- main pipeline ----------
            phase_l0_ru(0, defer=False)
            h0_cur = h0i[:, :]
            h1_cur = h1i[:, :]
            next_l1 = 0

            def l1_step(tg):
                nonlocal h1_cur, next_l1
                c1, tl1 = divmod(tg, LCH)
                h1_new = cellp.tile([HID, BL], bf16, tag="h1s")
                h1_cur = cell(1, c1, tl1, h1_cur, h1_new[:, :])
                next_l1 = tg + 1

            for c in range(nch):
                phase_l0_o(c)
                if c + 1 < nch:
                    phase_l0_ru(c + 1)
                hs = seqp.tile([HID, LCH * BL], bf16, tag="h0seq")
                h0seq[c] = hs
                for tl in range(chlen[c]):
                    h0_cur = cell(0, c, tl, h0_cur, hs[:, tl * BL:(tl + 1) * BL])
                    # the previous chunk's queued L1 phase matmuls must all be
                    # emitted before the first l1_step that reads their banks
                    drain_pending(5 if tl == 0 else 2)
                    tg1 = c * LCH + tl - LCH
                    if tg1 >= 0:
                        l1_step(tg1)
                phase_l1(c)

            drain_pending(len(pending))
            for tg1 in range(next_l1, seq_t):
                l1_step(tg1)

            # ---------- FC ----------
            out_sb = outp.tile([BL, NCLS], fp32, tag="osb")
            nsl = [512, 512, 512, NCLS - 3 * 512]
            for i in range(4):
                n0 = i * 512
                fc = ru1ps.tile([BL, 512], fp32, tag="ru1")
                pf = fc[:, 0:nsl[i]]
                nc.tensor.matmul(pf, ones_sb[:, 0:BL], bfc_sb[:, n0:n0 + nsl[i]],
                                 start=True, stop=False)
                nc.tensor.matmul(pf, h1_cur, wfc_sb[:, n0:n0 + nsl[i]],
                                 start=False, stop=True)
                nc.scalar.activation(out_sb[:, n0:n0 + nsl[i]], pf, AF.Identity)
            nc.sync.dma_start(out=OUT[:, :], in_=out_sb)

    nc.finalize()
    return nc


def _prep_consts(inputs):
    bf = ml_dtypes.bfloat16
    # L0 x-weights with the bias row appended (matches the ones-channel
    # appended to X's last K-tile)
    Wx0 = np.ascontiguousarray(np.concatenate([
        np.concatenate([inputs["Wr0"][:IN_CH], inputs["Wu0"][:IN_CH],
                        inputs["Wo0"][:IN_CH]], axis=1),
        np.concatenate([inputs["br0"], inputs["bu0"], inputs["bo0"]])[None, :],
    ], axis=0).astype(bf))
    Uh0 = np.ascontiguousarray(np.concatenate(
        [inputs["Wr0"][IN_CH:], inputs["Wu0"][IN_CH:], inputs["Wo0"][IN_CH:]],
        axis=1).astype(bf))
    Wx1 = np.ascontiguousarray(np.concatenate(
        [inputs["Wr1"][:HID], inputs["Wu1"][:HID], inputs["Wo1"][:HID]],
        axis=1).astype(bf))
    Uh1 = np.ascontiguousarray(np.concatenate(
        [inputs["Wr1"][HID:], inputs["Wu1"][HID:], inputs["Wo1"][HID:]],
        axis=1).astype(bf))
    B1R = np.ascontiguousarray(np.concatenate(
        [inputs["br1"], inputs["bu1"], inputs["bo1"]])[None, :].astype(bf))
    B1O = np.ascontiguousarray(inputs["bo1"][:, None].astype(np.float32))
    WFC = np.ascontiguousarray(inputs["Wfc"].astype(bf))
    BFC = np.ascontiguousarray(inputs["bfc"][None, :].astype(bf))
    return dict(WX0=Wx0, UH0=Uh0, WX1=Wx1, UH1=Uh1, B1R=B1R, B1O=B1O,
                WFC=WFC, BFC=BFC)


def kernel(_trace=False, **inputs):
    from concourse.bass_utils import run_bass_kernel_spmd

    seq_t = inputs["X"].shape[2]
    if "nc" not in _CACHE or _CACHE.get("seq_t") != seq_t:
        _CACHE["nc"] = _build(seq_t)
        _CACHE["seq_t"] = seq_t
    nc = _CACHE["nc"]

    consts = _prep_consts(inputs)
    bf = ml_dtypes.bfloat16
    # [B, C, T] -> per-core [C, T, BL] (t-major columns: col = t*BL + b)
    X = inputs["X"].astype(bf)
    in_maps = []
    for c in range(NCORES):
        m = dict(consts)
        xc = X[c * BL:(c + 1) * BL].transpose(1, 2, 0)  # [C, T, BL]
        m["XT"] = np.ascontiguousarray(xc).reshape(IN_CH, seq_t * BL)
        in_maps.append(m)

    res = run_bass_kernel_spmd(nc, in_maps, core_ids=list(range(NCORES)),
                               trace=_trace)
    out = np.concatenate([r["OUT"] for r in res.results], axis=0)
    if _trace:
        _CACHE["last_exec_time_ns"] = res.exec_time_ns
        _CACHE["last_profile"] = res.profile_json
    return out
